# revision 28
# baseline (speedup 1.0000x reference)
"""Trainium2 kernel for nn_BeamCharacterDecoder: CTC-style beam decode over
logits [T=128, B=8, C=25000], beam width 4.

Math: the reference adds the per-beam score to the logits before softmax; a
per-row constant doesn't change softmax, so every beam sees the same prob row
and (verified bit-exact against the reference) the whole scan collapses to
per-(t, b) row statistics over C:

  t = 0 : top-4 (prob, index) of softmax(logits[0, b])  -> the 4 beams
  t >= 1: every beam appends argmax_c logits[t, b] and multiplies by the max
          prob; beam order is preserved by the per-step re-sort and the final
          sort is a stable no-op (final scores are identical across beams).

So the device only needs, per (t, b): row max, argmax (first index on ties),
and sum(exp(x - max)). Sharding: data-parallel over B, one batch element per
NeuronCore. Each core streams its [128, 25000] f32 shard (12.8 MB, the memory
roofline) in 8 column chunks and emits a hierarchical reduction:

  bm [128, 200] f32  per-125-element sub-block maxes   (DVE, one pass)
  s  [128, 8]   f32  per-chunk sums of exp(x - 5)          (ACT exp+accum)

The host merge is tiny: global max/argmax from bm (+ a 125-element rescan of
the winning sub-block, matching the reference's lowest-index tie-break),
sumexp = exp(5 - M) * sum_k s_k, then closed-form assembly of
(seqs, scores, logs).
"""

import numpy as np

T = 128
B = 8
C = 25000
W = 4            # beam width
BLANK = 0
EOS = 1

SB = 125         # sub-block width of the max hierarchy
# Chunk sizes (elems): with fp16 input the DMA stream runs ~2x faster than
# the compute engines, so the reduce/exp chains are the critical path. A tiny
# FIRST chunk starts them as early as possible; a tiny last chunk keeps the
# final lag short; the engines never starve in between. Multiples of SB.
CHUNKS = [375, 4000, 4250, 4250, 4250, 4000, 2500, 1000, 375]
assert sum(CHUNKS) == C and all(c % SB == 0 for c in CHUNKS)
K = len(CHUNKS)
NB = C // SB     # 200 sub-blocks per row
CH_OFF = [sum(CHUNKS[:i]) for i in range(K + 1)]      # element offsets
NB_OFF = [o // SB for o in CH_OFF]                    # sub-block offsets

_NC = None


def _build_nc():
    from contextlib import ExitStack

    import concourse.tile as tile
    from concourse import bacc, mybir

    F32 = mybir.dt.float32
    F16 = mybir.dt.float16
    nc = bacc.Bacc(
        "TRN2",
        target_bir_lowering=False,
        debug=False,
        enable_asserts=False,
    )
    # Input is float16 (host-converted): halves the DMA stream, which was the
    # bottleneck. Quantization is monotonic, so the true fp32 argmax always
    # lives in a sub-block whose fp16 max ties the global fp16 max — the host
    # rescans those blocks in fp32 for exact indices and the exact row max.
    # sumexp from fp16 inputs measures 1.8e-5 worst-case relative error
    # (round-to-nearest errors cancel over 25000 terms).
    # Layout is chunk-major (each chunk's [T, ch] block contiguous) so every
    # DMA job is one fully linear HBM read; strided reads hit a bimodal +6us
    # contention mode on the shared HBM domain.
    x = nc.dram_tensor("x", [T * C], F16, kind="ExternalInput").ap()
    # Single merged output [T, NB+K]: cols 0..NB = sub-block maxes, cols
    # NB..NB+K = per-chunk exp sums. One DMA job + one completion wait at the
    # tail instead of two (~1us).
    o_out = nc.dram_tensor("o", [T, NB + K], F32, kind="ExternalOutput").ap()

    with tile.TileContext(nc) as tc, ExitStack() as ctx:
        # Whole shard stays resident (100 KB/partition): every chunk gets its
        # own tile so all input DMAs issue up-front with no recycling deps.
        inp = ctx.enter_context(tc.tile_pool(name="inp", bufs=1))
        scr = ctx.enter_context(tc.tile_pool(name="scr", bufs=1))
        stats = ctx.enter_context(tc.tile_pool(name="stats", bufs=1))

        o = stats.tile([T, NB + K], F32)
        bm, s = o[:, :NB], o[:, NB:]
        expscr = scr.tile([T, max(CHUNKS)], F32)

        # Constant exp bias: exp(x - 5) never over/underflows for N(0,1)
        # logits, and a constant bias means the exp stream depends only on
        # the DMA, never on the reduce stream (host rescales by exp(5 - M)).
        nbias = stats.tile([T, 1], F32)
        nc.gpsimd.memset(nbias[:], -5.0)
        # Dummy exp to pull ACT_TABLE_LOAD (~1.3us) into the DMA fill window
        # instead of delaying the first real exp.
        warm = stats.tile([T, 1], F32)
        nc.scalar.activation(warm[:], nbias[:], mybir.ActivationFunctionType.Exp)

        for k in range(K):
            ch = CHUNKS[k]
            t = inp.tile([T, ch], F16, tag=f"chunk{k}")
            src = x[T * CH_OFF[k] : T * CH_OFF[k + 1]].rearrange("(p n) -> p n", n=ch)
            nc.sync.dma_start(t[:], src)
            nc.vector.reduce_max(
                bm[:, NB_OFF[k] : NB_OFF[k + 1]],
                t[:].rearrange("p (n sb) -> p n sb", sb=SB),
                axis=mybir.AxisListType.X,
            )
            nc.scalar.activation(
                expscr[:, :ch],
                t[:],
                mybir.ActivationFunctionType.Exp,
                bias=nbias[:],
                accum_out=s[:, k : k + 1],
            )
        # Output on sync's HWDGE — it is idle by the time this is ready.
        nc.sync.dma_start(o_out, o[:])

    nc.compile()
    return nc


def _get_nc():
    global _NC
    if _NC is None:
        _NC = _build_nc()
    return _NC


def _run_device(logits, trace=False):
    from concourse.bass_utils import run_bass_kernel_spmd

    in_maps = []
    for b in range(B):
        xb = logits[:, b, :].astype(np.float16)
        in_maps.append(
            {
                "x": np.concatenate(
                    [xb[:, CH_OFF[k] : CH_OFF[k + 1]].ravel() for k in range(K)]
                )
            }
        )
    return run_bass_kernel_spmd(_get_nc(), in_maps, core_ids=list(range(B)), trace=trace)


def _postprocess(logits, results):
    """Tiny host merge of the per-core hierarchical reductions into the
    reference's (seqs, scores, logs)."""
    seqs = np.zeros((B, W, T + 1), np.int32)
    scores = np.zeros((B, W), np.float32)
    logs = np.zeros((B, W), np.float32)

    for b in range(B):
        o = results[b]["o"]            # [T, NB+K] f32
        bm, s = o[:, :NB], o[:, NB:]   # sub-block maxes / per-chunk exp sums
        xb = logits[:, b, :]           # [T, C]  f32 (host copy of the input)

        # bm holds fp16-quantized sub-block maxes; quantization is monotonic,
        # so the true fp32 row max lives in a block tying the global q-max.
        # Rescan those blocks (almost always exactly one) in fp32 for the
        # exact max and its first index.
        qmax = bm.max(axis=1)          # [T]
        M = np.empty(T, np.float32)
        amax = np.empty(T, np.int64)
        for t in range(T):
            best_v, best_i = -np.inf, -1
            for blk in np.flatnonzero(bm[t] == qmax[t]):
                seg = xb[t, blk * SB : (blk + 1) * SB]
                i = int(np.argmax(seg))
                if seg[i] > best_v:
                    best_v, best_i = seg[i], blk * SB + i
            M[t] = best_v
            amax[t] = best_i

        # sumexp merge: the device accumulated sum_c exp(x - 5) per chunk;
        # rescale to sum_c exp(x - M) in float64 then cast.
        sumexp = (
            s.astype(np.float64).sum(axis=1)
            * np.exp(5.0 - M.astype(np.float64))
        ).astype(np.float32)

        pmax = (np.float32(1.0) / sumexp).astype(np.float32)   # [T]
        logpmax = np.log(pmax, dtype=np.float32)

        # t = 0: top-4 characters (value-desc, index-asc on ties)
        row = xb[0]
        cand = np.argpartition(-row, W + 4)[: W + 4]
        cand = cand[np.lexsort((cand, -row[cand]))][:W]
        p0 = (
            np.exp((row[cand] - M[0]).astype(np.float32), dtype=np.float32) / sumexp[0]
        ).astype(np.float32)
        logp0 = np.log(p0, dtype=np.float32)

        # logs: sequential fp32 accumulation, matching the reference's scan
        acc = logp0.copy()
        for t in range(1, T):
            acc = (acc + logpmax[t]).astype(np.float32)
        logs[b] = acc
        scores[b] = pmax[T - 1]

        seqs[b, :, 1] = np.where(cand == EOS, BLANK, cand).astype(np.int32)
        ch = np.where(amax == EOS, BLANK, amax).astype(np.int32)   # [T]
        seqs[b, :, 2:] = ch[1:][None, :]

    return seqs, scores, logs


def kernel(logits, seq_len):
    logits = np.asarray(logits, dtype=np.float32)
    res = _run_device(logits)
    return _postprocess(logits, res.results)


# revision 30
# speedup vs baseline: 1.0365x; 1.0365x over previous
"""Trainium2 kernel for nn_BeamCharacterDecoder: CTC-style beam decode over
logits [T=128, B=8, C=25000], beam width 4.

Math: the reference adds the per-beam score to the logits before softmax; a
per-row constant doesn't change softmax, so every beam sees the same prob row
and (verified bit-exact against the reference) the whole scan collapses to
per-(t, b) row statistics over C:

  t = 0 : top-4 (prob, index) of softmax(logits[0, b])  -> the 4 beams
  t >= 1: every beam appends argmax_c logits[t, b] and multiplies by the max
          prob; beam order is preserved by the per-step re-sort and the final
          sort is a stable no-op (final scores are identical across beams).

So the device only needs, per (t, b): row max, argmax (first index on ties),
and sum(exp(x - max)). Sharding: data-parallel over B, one batch element per
NeuronCore. Each core streams its [128, 25000] f32 shard (12.8 MB, the memory
roofline) in 8 column chunks and emits a hierarchical reduction:

  bm [128, 200] f32  per-125-element sub-block maxes   (DVE, one pass)
  s  [128, 8]   f32  per-chunk sums of exp(x - 5)          (ACT exp+accum)

The host merge is tiny: global max/argmax from bm (+ a 125-element rescan of
the winning sub-block, matching the reference's lowest-index tie-break),
sumexp = exp(5 - M) * sum_k s_k, then closed-form assembly of
(seqs, scores, logs).
"""

import numpy as np

T = 128
B = 8
C = 25000
W = 4            # beam width
BLANK = 0
EOS = 1

SB = 125         # sub-block width of the max hierarchy
# Chunk sizes (elems): measured-best taper. With fp16 input the DMA outruns
# the compute engines, and the Vector reduce chain is the critical path;
# fewer chunks keep its per-instruction overhead low (a tiny-first-chunk
# early-start variant measured ~1us slower). Multiples of SB.
CHUNKS = [3625, 4000, 4000, 4000, 4000, 3500, 1500, 375]
assert sum(CHUNKS) == C and all(c % SB == 0 for c in CHUNKS)
K = len(CHUNKS)
NB = C // SB     # 200 sub-blocks per row
CH_OFF = [sum(CHUNKS[:i]) for i in range(K + 1)]      # element offsets
NB_OFF = [o // SB for o in CH_OFF]                    # sub-block offsets

_NC = None


def _build_nc():
    from contextlib import ExitStack

    import concourse.tile as tile
    from concourse import bacc, mybir

    F32 = mybir.dt.float32
    F16 = mybir.dt.float16
    nc = bacc.Bacc(
        "TRN2",
        target_bir_lowering=False,
        debug=False,
        enable_asserts=False,
    )
    # Input is float16 (host-converted): halves the DMA stream, which was the
    # bottleneck. Quantization is monotonic, so the true fp32 argmax always
    # lives in a sub-block whose fp16 max ties the global fp16 max — the host
    # rescans those blocks in fp32 for exact indices and the exact row max.
    # sumexp from fp16 inputs measures 1.8e-5 worst-case relative error
    # (round-to-nearest errors cancel over 25000 terms).
    # Layout is chunk-major (each chunk's [T, ch] block contiguous) so every
    # DMA job is one fully linear HBM read; strided reads hit a bimodal +6us
    # contention mode on the shared HBM domain.
    x = nc.dram_tensor("x", [T * C], F16, kind="ExternalInput").ap()
    # Single merged output [T, NB+K]: cols 0..NB = sub-block maxes, cols
    # NB..NB+K = per-chunk exp sums. One DMA job + one completion wait at the
    # tail instead of two (~1us).
    o_out = nc.dram_tensor("o", [T, NB + K], F32, kind="ExternalOutput").ap()

    with tile.TileContext(nc) as tc, ExitStack() as ctx:
        # Whole shard stays resident (100 KB/partition): every chunk gets its
        # own tile so all input DMAs issue up-front with no recycling deps.
        inp = ctx.enter_context(tc.tile_pool(name="inp", bufs=1))
        scr = ctx.enter_context(tc.tile_pool(name="scr", bufs=1))
        stats = ctx.enter_context(tc.tile_pool(name="stats", bufs=1))

        o = stats.tile([T, NB + K], F32)
        bm, s = o[:, :NB], o[:, NB:]
        expscr = scr.tile([T, max(CHUNKS)], F32)

        # Constant exp bias: exp(x - 5) never over/underflows for N(0,1)
        # logits, and a constant bias means the exp stream depends only on
        # the DMA, never on the reduce stream (host rescales by exp(5 - M)).
        nbias = stats.tile([T, 1], F32)
        nc.gpsimd.memset(nbias[:], -5.0)

        for k in range(K):
            ch = CHUNKS[k]
            t = inp.tile([T, ch], F16, tag=f"chunk{k}")
            src = x[T * CH_OFF[k] : T * CH_OFF[k + 1]].rearrange("(p n) -> p n", n=ch)
            nc.sync.dma_start(t[:], src)
            nc.vector.reduce_max(
                bm[:, NB_OFF[k] : NB_OFF[k + 1]],
                t[:].rearrange("p (n sb) -> p n sb", sb=SB),
                axis=mybir.AxisListType.X,
            )
            nc.scalar.activation(
                expscr[:, :ch],
                t[:],
                mybir.ActivationFunctionType.Exp,
                bias=nbias[:],
                accum_out=s[:, k : k + 1],
            )
        # Output on sync's HWDGE — it is idle by the time this is ready.
        nc.sync.dma_start(o_out, o[:])

    nc.compile()
    return nc


def _get_nc():
    global _NC
    if _NC is None:
        _NC = _build_nc()
    return _NC


def _run_device(logits, trace=False):
    from concourse.bass_utils import run_bass_kernel_spmd

    in_maps = []
    for b in range(B):
        xb = logits[:, b, :].astype(np.float16)
        in_maps.append(
            {
                "x": np.concatenate(
                    [xb[:, CH_OFF[k] : CH_OFF[k + 1]].ravel() for k in range(K)]
                )
            }
        )
    return run_bass_kernel_spmd(_get_nc(), in_maps, core_ids=list(range(B)), trace=trace)


def _postprocess(logits, results):
    """Tiny host merge of the per-core hierarchical reductions into the
    reference's (seqs, scores, logs)."""
    seqs = np.zeros((B, W, T + 1), np.int32)
    scores = np.zeros((B, W), np.float32)
    logs = np.zeros((B, W), np.float32)

    for b in range(B):
        o = results[b]["o"]            # [T, NB+K] f32
        bm, s = o[:, :NB], o[:, NB:]   # sub-block maxes / per-chunk exp sums
        xb = logits[:, b, :]           # [T, C]  f32 (host copy of the input)

        # bm holds fp16-quantized sub-block maxes; quantization is monotonic,
        # so the true fp32 row max lives in a block tying the global q-max.
        # Rescan those blocks (almost always exactly one) in fp32 for the
        # exact max and its first index.
        qmax = bm.max(axis=1)          # [T]
        M = np.empty(T, np.float32)
        amax = np.empty(T, np.int64)
        for t in range(T):
            best_v, best_i = -np.inf, -1
            for blk in np.flatnonzero(bm[t] == qmax[t]):
                seg = xb[t, blk * SB : (blk + 1) * SB]
                i = int(np.argmax(seg))
                if seg[i] > best_v:
                    best_v, best_i = seg[i], blk * SB + i
            M[t] = best_v
            amax[t] = best_i

        # sumexp merge: the device accumulated sum_c exp(x - 5) per chunk;
        # rescale to sum_c exp(x - M) in float64 then cast.
        sumexp = (
            s.astype(np.float64).sum(axis=1)
            * np.exp(5.0 - M.astype(np.float64))
        ).astype(np.float32)

        pmax = (np.float32(1.0) / sumexp).astype(np.float32)   # [T]
        logpmax = np.log(pmax, dtype=np.float32)

        # t = 0: top-4 characters (value-desc, index-asc on ties)
        row = xb[0]
        cand = np.argpartition(-row, W + 4)[: W + 4]
        cand = cand[np.lexsort((cand, -row[cand]))][:W]
        p0 = (
            np.exp((row[cand] - M[0]).astype(np.float32), dtype=np.float32) / sumexp[0]
        ).astype(np.float32)
        logp0 = np.log(p0, dtype=np.float32)

        # logs: sequential fp32 accumulation, matching the reference's scan
        acc = logp0.copy()
        for t in range(1, T):
            acc = (acc + logpmax[t]).astype(np.float32)
        logs[b] = acc
        scores[b] = pmax[T - 1]

        seqs[b, :, 1] = np.where(cand == EOS, BLANK, cand).astype(np.int32)
        ch = np.where(amax == EOS, BLANK, amax).astype(np.int32)   # [T]
        seqs[b, :, 2:] = ch[1:][None, :]

    return seqs, scores, logs


def kernel(logits, seq_len):
    logits = np.asarray(logits, dtype=np.float32)
    res = _run_device(logits)
    return _postprocess(logits, res.results)
